# revision 24
# baseline (speedup 1.0000x reference)
"""GAT (2-layer, PyG-style) Trainium2 Bass kernel, 8-core SPMD. v3.

Strategy (dst-sharded edge aggregation, fp16 edge path):
- Host sorts edges by dst into 128-node blocks; within a block edges are
  ordered [src<32768 | src>=32768], each group padded to a multiple of 128
  (MLO/MHI chunks of 128 edge slots, global maxima). Core c owns blocks
  [c*BPC, (c+1)*BPC) and produces output rows for its own nodes only.
- Node tables live in DRAM with rows padded to 256B multiples so that
  InstDMAGatherAnt (int16 indices, one descriptor per edge, ~1us fixed cost
  per call) batches a whole block's gathers into 2 calls (low/high rows).
- a_dst is gathered from small per-core local tables whose indices fit
  int16: t1loc (written by phase 0) for layer 1, cc_in itself for layer 2.
- Edge softmax + scatter-add are expressed via one-hot S matrices + PE
  matmuls (fp16 in, fp32 PSUM accumulate). Logits are exp-shifted by -2
  (cancels exactly in softmax; keeps fp16 exp finite).
- ELU's "-1" is dropped (eluplus = relu(x)+exp(min(x,0))) and corrected at
  the end: out -= colsum(W2) (exact since softmax weights sum to 1); the
  induced constant layer-2 logit shift c0 is subtracted pre-leaky_relu.
- One AllGather of the 128-col fp16 layer-2 table is the only collective.
"""

import numpy as np

import concourse.bacc as bacc
import concourse.bass as bass
import concourse.mybir as mybir
import concourse.tile as tile
from concourse import library_config
from concourse.bass_utils import run_bass_kernel_spmd
from bass_rust import add_dep_helper


def _dep(a, b, reason):
    ia = a.ins if hasattr(a, "ins") else a
    ib = b.ins if hasattr(b, "ins") else b
    add_dep_helper(ia, ib, reason=reason)


P = 128
NCORES = 8
EPS = 1e-16
NEG_SLOPE = 0.2
LO = 32768                  # int16 index limit -> low/high table split
F32 = mybir.dt.float32
F16 = mybir.dt.float16
I32 = mybir.dt.int32
I16 = mybir.dt.int16
AF = mybir.ActivationFunctionType
ALU = mybir.AluOpType


class Cfg:
    def __init__(self, n_nodes, mlo, mhi, c_in=128, h1=8, ch1=32, c2=64,
                 ncores=NCORES):
        self.n = n_nodes
        self.c_in = c_in
        self.h1 = h1
        self.ch1 = ch1
        self.hc1 = h1 * ch1          # 256
        self.c2 = c2
        self.ncores = ncores
        self.bpc = -(-n_nodes // (P * ncores))      # 49
        self.npad = ncores * self.bpc * P
        self.nblk = ncores * self.bpc
        self.mlo = mlo
        self.mhi = mhi
        self.mb = mlo + mhi
        self.t1w = 384               # [h 256 | asrc 8 | adst 8 | pad]
        self.t2w = 128               # [h2 64 | asrc2 1 | adst2 1 | pad]
        self.nloc = ((self.bpc + 7) // 8) * 8 * P   # t1loc rows
        # eidx blob layout per block, in i32 columns:
        self.o_lo = 0                          # srclo idx16: mlo*4 i32 cols
        self.o_hi = self.o_lo + mlo * 4        # srchi idx16: mhi*4
        self.o_ad = self.o_hi + mhi * 4        # adst idx16:  mb*4
        self.o_dr = self.o_ad + self.mb * 4    # drel f32:    mb
        self.K = self.o_dr + self.mb



_GQ = [0]


def _gpieces(nc, dep_fn, out_tile, cbase, W, in_ap, idx16, nchunks, dep, why):
    """Emit dma_gathers in <=1024-index pieces, alternating SWDGE queues."""
    for k0 in range(0, nchunks, 8):
        nk = min(8, nchunks - k0)
        q = _GQ[0] % 4
        _GQ[0] += 1
        g = nc.gpsimd.dma_gather(
            out_ap=out_tile[:, (cbase + k0) * W:(cbase + k0 + nk) * W]
                .rearrange("p (m w) -> p m w", m=nk),
            in_ap=in_ap, idxs_ap=idx16[:, k0 * 8:(k0 + nk) * 8],
            num_idxs=nk * 128, num_idxs_reg=nk * 128, elem_size=W,
            queue_num=q)
        dep_fn(g, dep, why)


def build_program(cfg):
    nc = bacc.Bacc(None, num_devices=cfg.ncores, num_swdge_queues=4)
    HC1, H1, CH1, C2 = cfg.hc1, cfg.h1, cfg.ch1, cfg.c2
    T1W, T2W, BPC = cfg.t1w, cfg.t2w, cfg.bpc
    MLO, MHI, MB = cfg.mlo, cfg.mhi, cfg.mb
    NBLK, NPAD, NLOC = cfg.nblk, cfg.npad, cfg.nloc
    G1 = 8
    assert NBLK % G1 == 0
    G0 = NLOC // (G1 * P)

    # ---- I/O ----
    xt = nc.dram_tensor("xt", [cfg.c_in, NPAD], F16, kind="ExternalInput")
    w1aug = nc.dram_tensor("w1aug", [cfg.c_in, 272], F16, kind="ExternalInput")
    w2aug = nc.dram_tensor("w2aug", [HC1, 66], F16, kind="ExternalInput")
    b1b = nc.dram_tensor("b1b", [P, HC1], F16, kind="ExternalInput")
    b2b = nc.dram_tensor("b2b", [P, C2], F32, kind="ExternalInput")
    iota = nc.dram_tensor("iota", [P, P], F16, kind="ExternalInput")
    ident = nc.dram_tensor("ident", [P, P], F16, kind="ExternalInput")
    shifts = nc.dram_tensor("shifts", [P, 2], F32, kind="ExternalInput")
    xoff = nc.dram_tensor("xoff", [P, G0], I32, kind="ExternalInput")
    eidx = nc.dram_tensor("eidx", [BPC, P, cfg.K], I32, kind="ExternalInput")
    out = nc.dram_tensor("out", [BPC * P, C2], F32, kind="ExternalOutput")

    # ---- internal DRAM ----
    t1 = nc.dram_tensor("t1", [NPAD, T1W], F16)
    t1loc = nc.dram_tensor("t1loc", [NLOC, T2W], F16)
    cc_in = nc.dram_tensor("cc_in", [BPC * P, T2W], F16)
    t2 = nc.dram_tensor("t2", [NPAD, T2W], F16, addr_space="Shared")

    groups = [list(range(cfg.ncores))]

    with tile.TileContext(nc) as tc:
        with (
            tc.tile_pool(name="const", bufs=1) as cpool,
            tc.tile_pool(name="p1", bufs=3) as p1pool,
            tc.tile_pool(name="blk", bufs=2) as bpool,
            tc.tile_pool(name="s", bufs=6) as spool,
            tc.tile_pool(name="fin", bufs=3) as opool,
            tc.tile_pool(name="ps", bufs=2, space="PSUM") as ps,
        ):
            # ---------------- constants ----------------
            iota_s = cpool.tile([P, P], F16)
            nc.sync.dma_start(out=iota_s[:], in_=iota[:])
            ident_s = cpool.tile([P, P], F16)
            nc.sync.dma_start(out=ident_s[:], in_=ident[:])
            w1aug_s = cpool.tile([P, 272], F16)
            nc.sync.dma_start(out=w1aug_s[:], in_=w1aug[:])
            w2aug_s = []
            for j in range(HC1 // P):
                wg = cpool.tile([P, 66], F16, tag=f"w2aug{j}")
                nc.sync.dma_start(out=wg[:], in_=w2aug[j * P:(j + 1) * P, :])
                w2aug_s.append(wg)
            b1b_s = cpool.tile([P, HC1], F16)
            nc.sync.dma_start(out=b1b_s[:], in_=b1b[:])
            b2b_s = cpool.tile([P, C2], F32)
            nc.sync.dma_start(out=b2b_s[:], in_=b2b[:])
            shifts_s = cpool.tile([P, 2], F32)
            nc.sync.dma_start(out=shifts_s[:], in_=shifts[:])
            xoff_s = cpool.tile([P, G0], I32)
            nc.sync.dma_start(out=xoff_s[:], in_=xoff[:])

            # ------------- phase 0: local a_dst table (own blocks) -------
            loc_writes = []
            for g in range(G0):
                xg0 = p1pool.tile([P, G1 * P], F16, tag="xg0")
                nc.gpsimd.indirect_dma_start(
                    out=xg0[:], out_offset=None, in_=xt[:],
                    in_offset=bass.IndirectOffsetOnAxis(
                        ap=xoff_s[:, g:g + 1], axis=1))
                rows0 = p1pool.tile([P, G1 * H1], F16, tag="rows0")
                for j in range(G1):
                    pl = ps.tile([P, H1], F32, space="PSUM", tag="den")
                    nc.tensor.matmul(out=pl[:], lhsT=xg0[:, j * P:(j + 1) * P],
                                     rhs=w1aug_s[:, 264:272],
                                     start=True, stop=True)
                    nc.vector.tensor_scalar_add(
                        out=rows0[:, j * H1:(j + 1) * H1], in0=pl[:],
                        scalar1=0.0)
                loc_writes.append(nc.sync.dma_start(
                    out=t1loc[g * G1 * P:(g + 1) * G1 * P, 0:H1].rearrange(
                        "(j p) c -> p j c", j=G1),
                    in_=rows0[:].rearrange("p (j c) -> p j c", j=G1)))
            jloc_t = cpool.tile([1, 1], F32, tag="jloc")
            jloc = nc.gpsimd.memset(jloc_t[:], 0.0)
            for w in loc_writes:
                _dep(jloc, w, "adst gathers wait for local table")

            # ------------- phase 1: full node table (replicated) ---------
            t1_writes = []
            for grp in range(NBLK // G1):
                B0 = grp * G1
                xg = p1pool.tile([P, G1 * P], F16, tag="xg")
                nc.sync.dma_start(out=xg[:], in_=xt[:, B0 * P:(B0 + G1) * P])
                rows = p1pool.tile([P, G1 * 272], F16, tag="rows")
                for j in range(G1):
                    ph1 = ps.tile([P, 272], F32, space="PSUM", tag="acc")
                    nc.tensor.matmul(out=ph1[:], lhsT=xg[:, j * P:(j + 1) * P],
                                     rhs=w1aug_s[:], start=True, stop=True)
                    dst = rows[:, j * 272:(j + 1) * 272]
                    if j % 2 == 0:
                        nc.scalar.copy(out=dst, in_=ph1[:])
                    else:
                        nc.vector.tensor_scalar_add(out=dst, in0=ph1[:],
                                                    scalar1=0.0)
                t1_writes.append(nc.sync.dma_start(
                    out=t1[B0 * P:(B0 + G1) * P, 0:272].rearrange(
                        "(j p) c -> p j c", j=G1),
                    in_=rows[:].rearrange("p (j c) -> p j c", j=G1)))

            j1tile = cpool.tile([1, 1], F32, tag="j1")
            j1 = nc.gpsimd.memset(j1tile[:], 0.0)
            for w in t1_writes:
                _dep(j1, w, "layer1 gathers wait for full node table")

            # gpsimd ext-isa library containing InstDMAGatherAnt
            nc.gpsimd.load_library(library_config.mlp)

            # ------------- phase 2: layer-1 edge aggregation -------------
            cc_writes = []
            for b in range(BPC):
                r0 = b * P
                tlo = bpool.tile([P, MLO * 4], I32, tag="tlo")
                nc.sync.dma_start(out=tlo[:], in_=eidx[b, :, cfg.o_lo:cfg.o_hi])
                thi = bpool.tile([P, MHI * 4], I32, tag="thi")
                nc.sync.dma_start(out=thi[:], in_=eidx[b, :, cfg.o_hi:cfg.o_ad])
                tad = bpool.tile([P, MB * 4], I32, tag="tad")
                nc.sync.dma_start(out=tad[:], in_=eidx[b, :, cfg.o_ad:cfg.o_dr])
                tdr = bpool.tile([P, MB], I32, tag="tdr")
                nc.sync.dma_start(out=tdr[:], in_=eidx[b, :, cfg.o_dr:cfg.K])
                ilo = tlo[:].bitcast(I16)
                ihi = thi[:].bitcast(I16)
                iad = tad[:].bitcast(I16)
                drel = tdr[:].bitcast(F32)

                gath = bpool.tile([P, MB * T1W], F16, tag="gath")
                _gpieces(nc, _dep, gath, 0, T1W, t1[0:LO, :], ilo, MLO,
                         j1, "lo gather after table1")
                _gpieces(nc, _dep, gath, MLO, T1W, t1[LO:NPAD, :], ihi, MHI,
                         j1, "hi gather after table1")
                adstg = bpool.tile([P, MB * T2W], F16, tag="adstg")
                _gpieces(nc, _dep, adstg, 0, T2W, t1loc[:], iad, MB,
                         jloc, "adst gather after local table")

                gv = gath[:].rearrange("p (m w) -> p m w", m=MB)
                av = bpool.tile([P, MB * H1], F16, tag="av")
                nc.vector.tensor_tensor(
                    out=av[:].rearrange("p (m h) -> p m h", m=MB),
                    in0=gv[:, :, 256:264],
                    in1=adstg[:].rearrange("p (m w) -> p m w",
                                           m=MB)[:, :, 0:H1],
                    op=ALU.add)
                lk = bpool.tile([P, MB * H1], F16, tag="lk")
                nc.vector.scalar_tensor_tensor(
                    out=lk[:], in0=av[:], scalar=NEG_SLOPE, in1=av[:],
                    op0=ALU.mult, op1=ALU.max)
                efull = bpool.tile([P, MB * HC1], F16, tag="efull")
                nc.scalar.activation(
                    out=efull[:],
                    in_=lk[:].rearrange("p (m h) -> p m h", m=MB)
                          .to_broadcast([P, MB, H1, CH1]),
                    func=AF.Exp, bias=shifts_s[:, 0:1])
                wm = bpool.tile([P, MB * HC1], F16, tag="wm")
                nc.vector.tensor_tensor(
                    out=wm[:].rearrange("p (m c) -> p m c", m=MB),
                    in0=gv[:, :, 0:HC1],
                    in1=efull[:].rearrange("p (m c) -> p m c", m=MB),
                    op=ALU.mult)

                pacc = ps.tile([P, 272], F32, space="PSUM", tag="acc")
                pmsg = pacc[:, 0:HC1]
                pdent = ps.tile([P, H1], F32, space="PSUM", tag="den")
                pden = pdent[:]
                e4 = efull[:].rearrange("p (m h c) -> p m h c", m=MB, h=H1)
                for g in range(MB):
                    S = spool.tile([P, P], F16, tag="S")
                    nc.vector.tensor_scalar(
                        out=S[:], in0=iota_s[:], scalar1=drel[:, g:g + 1],
                        scalar2=None, op0=ALU.is_equal)
                    nc.tensor.matmul(out=pmsg, lhsT=S[:],
                                     rhs=wm[:, g * HC1:(g + 1) * HC1],
                                     start=(g == 0), stop=(g == MB - 1))
                    nc.tensor.matmul(out=pden, lhsT=S[:],
                                     rhs=e4[:, g, :, 0],
                                     start=(g == 0), stop=(g == MB - 1))

                den = opool.tile([P, H1], F32, tag="den")
                nc.vector.tensor_scalar_add(out=den[:], in0=pden,
                                            scalar1=EPS)
                rec = opool.tile([P, H1], F32, tag="rec")
                nc.vector.reciprocal(out=rec[:], in_=den[:])
                o1b = opool.tile([P, HC1], F16, tag="o1b")
                for h in range(H1):
                    sl = slice(h * CH1, (h + 1) * CH1)
                    nc.vector.scalar_tensor_tensor(
                        out=o1b[:, sl], in0=pacc[:, sl],
                        scalar=rec[:, h:h + 1], in1=b1b_s[:, sl],
                        op0=ALU.mult, op1=ALU.add)
                xn = opool.tile([P, HC1], F16, tag="xn")
                nc.gpsimd.tensor_scalar_min(out=xn[:], in0=o1b[:], scalar1=0.0)
                en = opool.tile([P, HC1], F16, tag="en")
                nc.scalar.activation(out=en[:], in_=xn[:], func=AF.Exp)
                helu = opool.tile([P, HC1], F16, tag="helu")
                nc.vector.scalar_tensor_tensor(
                    out=helu[:], in0=o1b[:], scalar=0.0, in1=en[:],
                    op0=ALU.max, op1=ALU.add)

                ph2 = ps.tile([P, 66], F32, space="PSUM", tag="ph2")
                for j in range(HC1 // P):
                    pT = ps.tile([P, P], F16, space="PSUM", tag="pT")
                    nc.tensor.transpose(out=pT[:],
                                        in_=helu[:, j * P:(j + 1) * P],
                                        identity=ident_s[:])
                    hT = opool.tile([P, P], F16, tag="hT")
                    nc.vector.tensor_scalar_add(out=hT[:], in0=pT[:],
                                                scalar1=0.0)
                    nc.tensor.matmul(out=ph2[:], lhsT=hT[:], rhs=w2aug_s[j][:],
                                     start=(j == 0), stop=(j == HC1 // P - 1))
                h2row = opool.tile([P, T2W], F16, tag="h2row")
                nc.vector.tensor_scalar_add(out=h2row[:, 0:66], in0=ph2[:],
                                            scalar1=0.0)
                nc.vector.memset(h2row[:, 66:T2W], 0.0)
                cc_writes.append(nc.sync.dma_start(
                    out=cc_in[r0:r0 + P, :], in_=h2row[:]))

            # ------------- phase 3: share layer-2 node table -------------
            nc.gpsimd.load_library(library_config.standard)
            cc = nc.gpsimd.collective_compute(
                "AllGather", ALU.bypass, replica_groups=groups,
                ins=[cc_in[:]], outs=[t2[:]])
            for w in cc_writes:
                _dep(cc, w, "allgather after cc writes")
            j2tile = cpool.tile([1, 1], F32, tag="j2")
            j2 = nc.gpsimd.memset(j2tile[:], 0.0)
            _dep(j2, cc, "layer2 gathers after allgather")
            nc.gpsimd.load_library(library_config.mlp)

            # ------------- phase 4: layer-2 edge aggregation -------------
            for b in range(BPC):
                r0 = b * P
                tlo = bpool.tile([P, MLO * 4], I32, tag="tlo2")
                nc.sync.dma_start(out=tlo[:], in_=eidx[b, :, cfg.o_lo:cfg.o_hi])
                thi = bpool.tile([P, MHI * 4], I32, tag="thi2")
                nc.sync.dma_start(out=thi[:], in_=eidx[b, :, cfg.o_hi:cfg.o_ad])
                tad = bpool.tile([P, MB * 4], I32, tag="tad2")
                nc.sync.dma_start(out=tad[:], in_=eidx[b, :, cfg.o_ad:cfg.o_dr])
                tdr = bpool.tile([P, MB], I32, tag="tdr2")
                nc.sync.dma_start(out=tdr[:], in_=eidx[b, :, cfg.o_dr:cfg.K])
                ilo = tlo[:].bitcast(I16)
                ihi = thi[:].bitcast(I16)
                iad = tad[:].bitcast(I16)
                drel = tdr[:].bitcast(F32)

                gath2 = bpool.tile([P, MB * T2W], F16, tag="gath2")
                _gpieces(nc, _dep, gath2, 0, T2W, t2[0:LO, :], ilo, MLO,
                         j2, "lo gather after table2")
                _gpieces(nc, _dep, gath2, MLO, T2W, t2[LO:NPAD, :], ihi, MHI,
                         j2, "hi gather after table2")
                adst2 = bpool.tile([P, MB * T2W], F16, tag="adst2")
                _gpieces(nc, _dep, adst2, 0, T2W, cc_in[:], iad, MB,
                         j2, "adst2 gather after cc writes")

                qv = gath2[:].rearrange("p (m w) -> p m w", m=MB)
                av2 = bpool.tile([P, MB], F16, tag="av2")
                nc.vector.scalar_tensor_tensor(
                    out=av2[:].rearrange("p (m o) -> p m o", m=MB),
                    in0=qv[:, :, 64:65], scalar=shifts_s[:, 1:2],
                    in1=adst2[:].rearrange("p (m w) -> p m w",
                                           m=MB)[:, :, 65:66],
                    op0=ALU.add, op1=ALU.add)
                lk2 = bpool.tile([P, MB], F16, tag="lk2")
                nc.vector.scalar_tensor_tensor(
                    out=lk2[:], in0=av2[:], scalar=NEG_SLOPE, in1=av2[:],
                    op0=ALU.mult, op1=ALU.max)
                e2full = bpool.tile([P, MB * C2], F16, tag="e2full")
                nc.scalar.activation(
                    out=e2full[:],
                    in_=lk2[:].rearrange("p (m o) -> p m o", m=MB)
                           .to_broadcast([P, MB, 1, C2]),
                    func=AF.Exp, bias=shifts_s[:, 0:1])
                wm2 = bpool.tile([P, MB * C2], F16, tag="wm2")
                nc.vector.tensor_tensor(
                    out=wm2[:].rearrange("p (m c) -> p m c", m=MB),
                    in0=qv[:, :, 0:C2],
                    in1=e2full[:].rearrange("p (m c) -> p m c", m=MB),
                    op=ALU.mult)

                pacc2 = ps.tile([P, 272], F32, space="PSUM", tag="acc")
                pmsg2 = pacc2[:, 0:C2]
                pdent2 = ps.tile([P, H1], F32, space="PSUM", tag="den")
                pden2 = pdent2[:, 0:1]
                for g in range(MB):
                    S = spool.tile([P, P], F16, tag="S")
                    nc.vector.tensor_scalar(
                        out=S[:], in0=iota_s[:], scalar1=drel[:, g:g + 1],
                        scalar2=None, op0=ALU.is_equal)
                    nc.tensor.matmul(out=pmsg2, lhsT=S[:],
                                     rhs=wm2[:, g * C2:(g + 1) * C2],
                                     start=(g == 0), stop=(g == MB - 1))
                    nc.tensor.matmul(out=pden2, lhsT=S[:],
                                     rhs=e2full[:, g * C2:g * C2 + 1],
                                     start=(g == 0), stop=(g == MB - 1))

                den2 = opool.tile([P, 1], F32, tag="den2")
                nc.vector.tensor_scalar_add(out=den2[:], in0=pden2,
                                            scalar1=EPS)
                rec2 = opool.tile([P, 1], F32, tag="rec2")
                nc.vector.reciprocal(out=rec2[:], in_=den2[:])
                o2 = opool.tile([P, C2], F32, tag="o2")
                nc.vector.scalar_tensor_tensor(
                    out=o2[:], in0=pmsg2, scalar=rec2[:, 0:1],
                    in1=b2b_s[:], op0=ALU.mult, op1=ALU.add)
                nc.sync.dma_start(out=out[r0:r0 + P, :], in_=o2[:])

    nc.compile()
    return nc


def _wrap16(idx, nid):
    """Pack an int16 index list (len nid) into a [128, nid//16] tile:
    element k at (k%16, k//16), replicated to partitions 16..127."""
    a = np.asarray(idx, np.int16).reshape(nid // 16, 16).T  # [16, nid//16]
    return np.tile(a, (8, 1))


def host_prep(cfg, edge_index):
    n = cfg.n
    src = np.asarray(edge_index[0]).astype(np.int64)
    dst = np.asarray(edge_index[1]).astype(np.int64)
    loop = np.arange(n, dtype=np.int64)
    src = np.concatenate([src, loop])
    dst = np.concatenate([dst, loop])

    order = np.argsort(dst, kind="stable")
    ss = src[order]
    ds = dst[order]
    blk = ds >> 7

    MLO, MHI, MB = cfg.mlo, cfg.mhi, cfg.mb
    NBLK, BPC = cfg.nblk, cfg.bpc
    eidx = np.zeros((NBLK, P, cfg.K), dtype=np.int32)

    starts = np.zeros(NBLK + 1, dtype=np.int64)
    np.cumsum(np.bincount(blk, minlength=NBLK), out=starts[1:])

    for B in range(NBLK):
        s_b = ss[starts[B]:starts[B + 1]]
        d_b = ds[starts[B]:starts[B + 1]]
        lo_m = s_b < LO
        s_lo, d_lo = s_b[lo_m], d_b[lo_m]
        s_hi, d_hi = s_b[~lo_m], d_b[~lo_m]
        nlo, nhi = len(s_lo), len(s_hi)
        assert -(-nlo // P) <= MLO and -(-nhi // P) <= MHI, (B, nlo, nhi)
        cbase = (B // BPC) * BPC * P

        ilo = np.zeros(MLO * P, np.int16)
        ilo[:nlo] = s_lo.astype(np.int16)
        ihi = np.zeros(MHI * P, np.int16)
        ihi[:nhi] = (s_hi - LO).astype(np.int16)
        kidx = np.concatenate([np.arange(nlo), MLO * P + np.arange(nhi)])
        d_all = np.concatenate([d_lo, d_hi])
        iad = np.zeros(MB * P, np.int64)
        iad[kidx] = d_all - cbase
        drel = np.full((P, MB), -1.0, dtype=np.float32)
        drel[kidx % P, kidx // P] = (d_all - (B << 7)).astype(np.float32)

        eidx[B, :, cfg.o_lo:cfg.o_hi] = np.ascontiguousarray(
            _wrap16(ilo, MLO * P)).view(np.int32)
        eidx[B, :, cfg.o_hi:cfg.o_ad] = np.ascontiguousarray(
            _wrap16(ihi, MHI * P)).view(np.int32)
        eidx[B, :, cfg.o_ad:cfg.o_dr] = np.ascontiguousarray(
            _wrap16(iad.astype(np.int16), MB * P)).view(np.int32)
        eidx[B, :, cfg.o_dr:cfg.K] = drel.view(np.int32)

    return [np.ascontiguousarray(eidx[c * BPC:(c + 1) * BPC])
            for c in range(cfg.ncores)]


def compute_m(n, edge_index):
    src = np.asarray(edge_index[0]).astype(np.int64)
    dst = np.asarray(edge_index[1]).astype(np.int64)
    loop = np.arange(n, dtype=np.int64)
    src = np.concatenate([src, loop])
    dst = np.concatenate([dst, loop])
    blk = dst >> 7
    nblk = -(-n // P)
    lo = src < LO
    clo = np.bincount(blk[lo], minlength=nblk)
    chi = np.bincount(blk[~lo], minlength=nblk)
    return int(-(-clo.max() // P)), int(-(-chi.max() // P))


def make_in_maps(cfg, x, W1, att_src1, att_dst1, bias1, W2, att_src2,
                 att_dst2, bias2, edge_index):
    H1, CH1, HC1, C2 = cfg.h1, cfg.ch1, cfg.hc1, cfg.c2
    x = np.asarray(x, dtype=np.float32)
    xpad = np.zeros((cfg.npad, cfg.c_in), dtype=np.float32)
    xpad[: cfg.n] = x
    xt = np.ascontiguousarray(xpad.T).astype(np.float16)

    W1 = np.asarray(W1, np.float32)
    W2 = np.asarray(W2, np.float32)
    as1 = np.asarray(att_src1, np.float32)
    ad1 = np.asarray(att_dst1, np.float32)
    as2 = np.asarray(att_src2, np.float32).reshape(-1)
    ad2 = np.asarray(att_dst2, np.float32).reshape(-1)

    A1s = np.zeros((HC1, H1), dtype=np.float32)
    A1d = np.zeros((HC1, H1), dtype=np.float32)
    hh = np.repeat(np.arange(H1), CH1)
    A1s[np.arange(HC1), hh] = as1.reshape(-1)
    A1d[np.arange(HC1), hh] = ad1.reshape(-1)
    w1aug = np.concatenate([W1, W1 @ A1s, W1 @ A1d], axis=1).astype(np.float16)
    w2aug = np.concatenate([W2, (W2 @ as2)[:, None], (W2 @ ad2)[:, None]],
                           axis=1).astype(np.float16)

    colsum = W2.sum(axis=0)
    c0 = float(colsum @ (as2 + ad2))
    shifts = np.zeros((P, 2), dtype=np.float32)
    shifts[:, 0] = -2.0   # exp bias (cancels in softmax; keeps fp16 safe)
    shifts[:, 1] = -c0    # undo eluplus fold's logit shift (pre-leaky)

    b1b = np.tile(np.asarray(bias1, np.float32).reshape(1, -1),
                  (P, 1)).astype(np.float16)
    b2b = np.tile((np.asarray(bias2, np.float32).reshape(-1) - colsum
                   ).reshape(1, -1), (P, 1)).astype(np.float32)
    iota = np.tile(np.arange(P, dtype=np.float16), (P, 1))
    ident = np.eye(P, dtype=np.float16)

    per_core = host_prep(cfg, edge_index)
    G0 = cfg.nloc // (8 * P)
    in_maps = []
    for c in range(cfg.ncores):
        m = {"xt": xt, "w1aug": w1aug, "w2aug": w2aug, "b1b": b1b,
             "b2b": b2b, "iota": iota, "ident": ident, "shifts": shifts}
        base = c * cfg.bpc * P
        cols = np.minimum(base + np.arange(G0) * 8 * P,
                          cfg.npad - 8 * P).astype(np.int64)
        xoff = (np.arange(P)[:, None] * cfg.npad +
                cols[None, :]).astype(np.int32)
        m["xoff"] = xoff
        m["eidx"] = per_core[c]
        in_maps.append(m)
    return in_maps


_prog_cache = {}
_last_results = None


def kernel(x, edge_index, edge_weight, W1, att_src1, att_dst1, bias1,
           W2, att_src2, att_dst2, bias2):
    global _last_results
    n = x.shape[0]
    # edge_weight is unused (GATConv with edge_dim=None ignores it)
    mlo, mhi = compute_m(n, edge_index)
    mlo, mhi = max(mlo, 13), max(mhi, 8)

    cfg = Cfg(n, mlo, mhi, c_in=x.shape[1], h1=8, ch1=32, c2=64)
    key = (cfg.n, cfg.c_in, cfg.mlo, cfg.mhi)
    if key not in _prog_cache:
        _prog_cache[key] = build_program(cfg)
    nc = _prog_cache[key]

    in_maps = make_in_maps(cfg, x, W1, att_src1, att_dst1, bias1, W2,
                           att_src2, att_dst2, bias2, edge_index)
    res = run_bass_kernel_spmd(nc, in_maps, list(range(cfg.ncores)))
    _last_results = res
    outs = [res.results[c]["out"] for c in range(cfg.ncores)]
    full = np.concatenate(outs, axis=0)[: cfg.n]
    return np.ascontiguousarray(full)


# revision 26
# speedup vs baseline: 1.2888x; 1.2888x over previous
"""GAT (2-layer, PyG-style) Trainium2 Bass kernel, 8-core SPMD. v3.

Strategy (dst-sharded edge aggregation, fp16 edge path):
- Host sorts edges by dst into 128-node blocks; within a block edges are
  ordered [src<32768 | src>=32768], each group padded to a multiple of 128
  (MLO/MHI chunks of 128 edge slots, global maxima). Core c owns blocks
  [c*BPC, (c+1)*BPC) and produces output rows for its own nodes only.
- Node tables live in DRAM with rows padded to 256B multiples so that
  InstDMAGatherAnt (int16 indices, one descriptor per edge, ~1us fixed cost
  per call) batches a whole block's gathers into 2 calls (low/high rows).
- a_dst is gathered from small per-core local tables whose indices fit
  int16: t1loc (written by phase 0) for layer 1, cc_in itself for layer 2.
- Edge softmax + scatter-add are expressed via one-hot S matrices + PE
  matmuls (fp16 in, fp32 PSUM accumulate). Logits are exp-shifted by -2
  (cancels exactly in softmax; keeps fp16 exp finite).
- ELU's "-1" is dropped (eluplus = relu(x)+exp(min(x,0))) and corrected at
  the end: out -= colsum(W2) (exact since softmax weights sum to 1); the
  induced constant layer-2 logit shift c0 is subtracted pre-leaky_relu.
- One AllGather of the 128-col fp16 layer-2 table is the only collective.
"""

import numpy as np

import concourse.bacc as bacc
import concourse.bass as bass
import concourse.mybir as mybir
import concourse.tile as tile
from concourse import library_config
from concourse.bass_utils import run_bass_kernel_spmd
from bass_rust import add_dep_helper


def _dep(a, b, reason):
    ia = a.ins if hasattr(a, "ins") else a
    ib = b.ins if hasattr(b, "ins") else b
    add_dep_helper(ia, ib, reason=reason)


P = 128
NCORES = 8
EPS = 1e-16
NEG_SLOPE = 0.2
LO = 32768                  # int16 index limit -> low/high table split
F32 = mybir.dt.float32
F16 = mybir.dt.float16
I32 = mybir.dt.int32
I16 = mybir.dt.int16
AF = mybir.ActivationFunctionType
ALU = mybir.AluOpType


class Cfg:
    def __init__(self, n_nodes, mlo, mhi, c_in=128, h1=8, ch1=32, c2=64,
                 ncores=NCORES, slot_lo=None, slot_hi=None):
        self.n = n_nodes
        self.c_in = c_in
        self.h1 = h1
        self.ch1 = ch1
        self.hc1 = h1 * ch1          # 256
        self.c2 = c2
        self.ncores = ncores
        self.bpc = -(-n_nodes // (P * ncores))      # 49
        self.npad = ncores * self.bpc * P
        self.nblk = ncores * self.bpc
        self.mlo = mlo
        self.mhi = mhi
        self.mb = mlo + mhi
        self.t1w = 384               # [h 256 | asrc 8 | adst 8 | pad]
        self.t2w = 128               # [h2 64 | asrc2 1 | adst2 1 | pad]
        self.nloc = ((self.bpc + 7) // 8) * 8 * P   # t1loc rows
        # eidx blob layout per block, in i32 columns:
        self.o_lo = 0                          # srclo idx16: mlo*4 i32 cols
        self.o_hi = self.o_lo + mlo * 4        # srchi idx16: mhi*4
        self.o_ad = self.o_hi + mhi * 4        # adst idx16:  mb*4
        self.o_dr = self.o_ad + self.mb * 4    # drel f32:    mb
        self.K = self.o_dr + self.mb
        # per-block-slot live chunk counts (max over cores; rest is padding)
        self.slot_lo = slot_lo if slot_lo is not None else [mlo] * self.bpc
        self.slot_hi = slot_hi if slot_hi is not None else [mhi] * self.bpc



_GQ = [0]


def _gpieces(nc, dep_fn, out_tile, cbase, W, in_ap, idx16, nchunks, dep, why):
    """Emit dma_gathers in <=1024-index pieces, alternating SWDGE queues."""
    for k0 in range(0, nchunks, 8):
        nk = min(8, nchunks - k0)
        q = _GQ[0] % 4
        _GQ[0] += 1
        g = nc.gpsimd.dma_gather(
            out_ap=out_tile[:, (cbase + k0) * W:(cbase + k0 + nk) * W]
                .rearrange("p (m w) -> p m w", m=nk),
            in_ap=in_ap, idxs_ap=idx16[:, k0 * 8:(k0 + nk) * 8],
            num_idxs=nk * 128, num_idxs_reg=nk * 128, elem_size=W,
            queue_num=q)
        dep_fn(g, dep, why)


def build_program(cfg):
    nc = bacc.Bacc(None, num_devices=cfg.ncores, num_swdge_queues=4)
    HC1, H1, CH1, C2 = cfg.hc1, cfg.h1, cfg.ch1, cfg.c2
    T1W, T2W, BPC = cfg.t1w, cfg.t2w, cfg.bpc
    MLO, MHI, MB = cfg.mlo, cfg.mhi, cfg.mb
    NBLK, NPAD, NLOC = cfg.nblk, cfg.npad, cfg.nloc
    G1 = 8
    assert NBLK % G1 == 0
    G0 = NLOC // (G1 * P)

    # ---- I/O ----
    xt = nc.dram_tensor("xt", [cfg.c_in, NPAD], F16, kind="ExternalInput")
    w1aug = nc.dram_tensor("w1aug", [cfg.c_in, 272], F16, kind="ExternalInput")
    w2aug = nc.dram_tensor("w2aug", [HC1, 66], F16, kind="ExternalInput")
    b1b = nc.dram_tensor("b1b", [P, HC1], F16, kind="ExternalInput")
    b2b = nc.dram_tensor("b2b", [P, C2], F32, kind="ExternalInput")
    iota = nc.dram_tensor("iota", [P, P], F16, kind="ExternalInput")
    ident = nc.dram_tensor("ident", [P, P], F16, kind="ExternalInput")
    shifts = nc.dram_tensor("shifts", [P, 2], F32, kind="ExternalInput")
    xoff = nc.dram_tensor("xoff", [P, G0], I32, kind="ExternalInput")
    eidx = nc.dram_tensor("eidx", [BPC, P, cfg.K], I32, kind="ExternalInput")
    out = nc.dram_tensor("out", [BPC * P, C2], F32, kind="ExternalOutput")

    # ---- internal DRAM ----
    t1 = nc.dram_tensor("t1", [NPAD, T1W], F16)
    t1loc = nc.dram_tensor("t1loc", [NLOC, T2W], F16)
    cc_in = nc.dram_tensor("cc_in", [BPC * P, T2W], F16)
    t2 = nc.dram_tensor("t2", [NPAD, T2W], F16, addr_space="Shared")

    groups = [list(range(cfg.ncores))]

    with tile.TileContext(nc) as tc:
        with (
            tc.tile_pool(name="const", bufs=1) as cpool,
            tc.tile_pool(name="p1", bufs=2) as p1pool,
            tc.tile_pool(name="blk", bufs=2) as bpool,
            tc.tile_pool(name="s", bufs=4) as spool,
            tc.tile_pool(name="fin", bufs=2) as opool,
            tc.tile_pool(name="ps", bufs=2, space="PSUM") as ps,
        ):
            # ---------------- constants ----------------
            iota_s = cpool.tile([P, P], F16)
            nc.sync.dma_start(out=iota_s[:], in_=iota[:])
            ident_s = cpool.tile([P, P], F16)
            nc.sync.dma_start(out=ident_s[:], in_=ident[:])
            w1aug_s = cpool.tile([P, 272], F16)
            nc.sync.dma_start(out=w1aug_s[:], in_=w1aug[:])
            w2aug_s = []
            for j in range(HC1 // P):
                wg = cpool.tile([P, 66], F16, tag=f"w2aug{j}")
                nc.sync.dma_start(out=wg[:], in_=w2aug[j * P:(j + 1) * P, :])
                w2aug_s.append(wg)
            b1b_s = cpool.tile([P, HC1], F16)
            nc.sync.dma_start(out=b1b_s[:], in_=b1b[:])
            b2b_s = cpool.tile([P, C2], F32)
            nc.sync.dma_start(out=b2b_s[:], in_=b2b[:])
            shifts_s = cpool.tile([P, 2], F32)
            nc.sync.dma_start(out=shifts_s[:], in_=shifts[:])
            xoff_s = cpool.tile([P, G0], I32)
            nc.sync.dma_start(out=xoff_s[:], in_=xoff[:])

            # ------------- phase 0: local a_dst table (own blocks) -------
            loc_writes = []
            for g in range(G0):
                xg0 = p1pool.tile([P, G1 * P], F16, tag="xg0")
                nc.gpsimd.indirect_dma_start(
                    out=xg0[:], out_offset=None, in_=xt[:],
                    in_offset=bass.IndirectOffsetOnAxis(
                        ap=xoff_s[:, g:g + 1], axis=1))
                rows0 = p1pool.tile([P, G1 * H1], F16, tag="rows0")
                for j in range(G1):
                    pl = ps.tile([P, H1], F32, space="PSUM", tag="den")
                    nc.tensor.matmul(out=pl[:], lhsT=xg0[:, j * P:(j + 1) * P],
                                     rhs=w1aug_s[:, 264:272],
                                     start=True, stop=True)
                    nc.vector.tensor_scalar_add(
                        out=rows0[:, j * H1:(j + 1) * H1], in0=pl[:],
                        scalar1=0.0)
                loc_writes.append(nc.sync.dma_start(
                    out=t1loc[g * G1 * P:(g + 1) * G1 * P, 0:H1].rearrange(
                        "(j p) c -> p j c", j=G1),
                    in_=rows0[:].rearrange("p (j c) -> p j c", j=G1)))
            jloc_t = cpool.tile([1, 1], F32, tag="jloc")
            jloc = nc.gpsimd.memset(jloc_t[:], 0.0)
            for w in loc_writes:
                _dep(jloc, w, "adst gathers wait for local table")

            # ------------- phase 1: full node table (replicated) ---------
            t1_writes = []
            for grp in range(NBLK // G1):
                B0 = grp * G1
                xg = p1pool.tile([P, G1 * P], F16, tag="xg")
                nc.sync.dma_start(out=xg[:], in_=xt[:, B0 * P:(B0 + G1) * P])
                rows = p1pool.tile([P, G1 * 272], F16, tag="rows")
                for j in range(G1):
                    ph1 = ps.tile([P, 272], F32, space="PSUM", tag="acc")
                    nc.tensor.matmul(out=ph1[:], lhsT=xg[:, j * P:(j + 1) * P],
                                     rhs=w1aug_s[:], start=True, stop=True)
                    dst = rows[:, j * 272:(j + 1) * 272]
                    if j % 2 == 0:
                        nc.scalar.copy(out=dst, in_=ph1[:])
                    else:
                        nc.vector.tensor_scalar_add(out=dst, in0=ph1[:],
                                                    scalar1=0.0)
                t1_writes.append(nc.sync.dma_start(
                    out=t1[B0 * P:(B0 + G1) * P, 0:272].rearrange(
                        "(j p) c -> p j c", j=G1),
                    in_=rows[:].rearrange("p (j c) -> p j c", j=G1)))

            j1tile = cpool.tile([1, 1], F32, tag="j1")
            j1 = nc.gpsimd.memset(j1tile[:], 0.0)
            for w in t1_writes:
                _dep(j1, w, "layer1 gathers wait for full node table")

            # gpsimd ext-isa library containing InstDMAGatherAnt
            nc.gpsimd.load_library(library_config.mlp)

            # ------------- phase 2: layer-1 edge aggregation -------------
            cc_writes = []
            for b in range(BPC):
                r0 = b * P
                tlo = bpool.tile([P, MLO * 4], I32, tag="tlo")
                nc.sync.dma_start(out=tlo[:], in_=eidx[b, :, cfg.o_lo:cfg.o_hi])
                thi = bpool.tile([P, MHI * 4], I32, tag="thi")
                nc.sync.dma_start(out=thi[:], in_=eidx[b, :, cfg.o_hi:cfg.o_ad])
                tad = bpool.tile([P, MB * 4], I32, tag="tad")
                nc.sync.dma_start(out=tad[:], in_=eidx[b, :, cfg.o_ad:cfg.o_dr])
                tdr = bpool.tile([P, MB], I32, tag="tdr")
                nc.sync.dma_start(out=tdr[:], in_=eidx[b, :, cfg.o_dr:cfg.K])
                ilo = tlo[:].bitcast(I16)
                ihi = thi[:].bitcast(I16)
                iad = tad[:].bitcast(I16)
                drel = tdr[:].bitcast(F32)

                nlo_c = cfg.slot_lo[b]
                nhi_c = cfg.slot_hi[b]
                glist = list(range(nlo_c)) + list(range(MLO, MLO + nhi_c))
                gath = bpool.tile([P, MB * T1W], F16, tag="gath")
                _gpieces(nc, _dep, gath, 0, T1W, t1[0:LO, :], ilo, nlo_c,
                         j1, "lo gather after table1")
                _gpieces(nc, _dep, gath, MLO, T1W, t1[LO:NPAD, :], ihi, nhi_c,
                         j1, "hi gather after table1")
                adstg = bpool.tile([P, MB * T2W], F16, tag="adstg")
                _gpieces(nc, _dep, adstg, 0, T2W, t1loc[:], iad, nlo_c,
                         jloc, "adst gather after local table")
                _gpieces(nc, _dep, adstg, MLO, T2W, t1loc[:],
                         iad[:, MLO * 8:], nhi_c,
                         jloc, "adst gather after local table")

                gv = gath[:].rearrange("p (m w) -> p m w", m=MB)
                av = bpool.tile([P, MB * H1], F16, tag="av")
                nc.vector.tensor_tensor(
                    out=av[:].rearrange("p (m h) -> p m h", m=MB),
                    in0=gv[:, :, 256:264],
                    in1=adstg[:].rearrange("p (m w) -> p m w",
                                           m=MB)[:, :, 0:H1],
                    op=ALU.add)
                lk = bpool.tile([P, MB * H1], F16, tag="lk")
                nc.vector.scalar_tensor_tensor(
                    out=lk[:], in0=av[:], scalar=NEG_SLOPE, in1=av[:],
                    op0=ALU.mult, op1=ALU.max)
                efull = bpool.tile([P, MB * HC1], F16, tag="efull")
                nc.scalar.activation(
                    out=efull[:],
                    in_=lk[:].rearrange("p (m h) -> p m h", m=MB)
                          .to_broadcast([P, MB, H1, CH1]),
                    func=AF.Exp, bias=shifts_s[:, 0:1])
                wm = bpool.tile([P, MB * HC1], F16, tag="wm")
                nc.vector.tensor_tensor(
                    out=wm[:].rearrange("p (m c) -> p m c", m=MB),
                    in0=gv[:, :, 0:HC1],
                    in1=efull[:].rearrange("p (m c) -> p m c", m=MB),
                    op=ALU.mult)

                pacc = ps.tile([P, 272], F32, space="PSUM", tag="acc")
                pmsg = pacc[:, 0:HC1]
                pdent = ps.tile([P, H1], F32, space="PSUM", tag="den")
                pden = pdent[:]
                e4 = efull[:].rearrange("p (m h c) -> p m h c", m=MB, h=H1)
                for gi, g in enumerate(glist):
                    S = spool.tile([P, P], F16, tag="S")
                    nc.vector.tensor_scalar(
                        out=S[:], in0=iota_s[:], scalar1=drel[:, g:g + 1],
                        scalar2=None, op0=ALU.is_equal)
                    nc.tensor.matmul(out=pmsg, lhsT=S[:],
                                     rhs=wm[:, g * HC1:(g + 1) * HC1],
                                     start=(gi == 0),
                                     stop=(gi == len(glist) - 1))
                    nc.tensor.matmul(out=pden, lhsT=S[:],
                                     rhs=e4[:, g, :, 0],
                                     start=(gi == 0),
                                     stop=(gi == len(glist) - 1))

                den = opool.tile([P, H1], F32, tag="den")
                nc.vector.tensor_scalar_add(out=den[:], in0=pden,
                                            scalar1=EPS)
                rec = opool.tile([P, H1], F32, tag="rec")
                nc.vector.reciprocal(out=rec[:], in_=den[:])
                o1b = opool.tile([P, HC1], F16, tag="o1b")
                for h in range(H1):
                    sl = slice(h * CH1, (h + 1) * CH1)
                    nc.vector.scalar_tensor_tensor(
                        out=o1b[:, sl], in0=pacc[:, sl],
                        scalar=rec[:, h:h + 1], in1=b1b_s[:, sl],
                        op0=ALU.mult, op1=ALU.add)
                xn = opool.tile([P, HC1], F16, tag="xn")
                nc.gpsimd.tensor_scalar_min(out=xn[:], in0=o1b[:], scalar1=0.0)
                en = opool.tile([P, HC1], F16, tag="en")
                nc.scalar.activation(out=en[:], in_=xn[:], func=AF.Exp)
                helu = opool.tile([P, HC1], F16, tag="helu")
                nc.vector.scalar_tensor_tensor(
                    out=helu[:], in0=o1b[:], scalar=0.0, in1=en[:],
                    op0=ALU.max, op1=ALU.add)

                ph2 = ps.tile([P, 66], F32, space="PSUM", tag="ph2")
                for j in range(HC1 // P):
                    pT = ps.tile([P, P], F16, space="PSUM", tag="pT")
                    nc.tensor.transpose(out=pT[:],
                                        in_=helu[:, j * P:(j + 1) * P],
                                        identity=ident_s[:])
                    hT = opool.tile([P, P], F16, tag="hT")
                    nc.vector.tensor_scalar_add(out=hT[:], in0=pT[:],
                                                scalar1=0.0)
                    nc.tensor.matmul(out=ph2[:], lhsT=hT[:], rhs=w2aug_s[j][:],
                                     start=(j == 0), stop=(j == HC1 // P - 1))
                h2row = opool.tile([P, T2W], F16, tag="h2row")
                nc.vector.tensor_scalar_add(out=h2row[:, 0:66], in0=ph2[:],
                                            scalar1=0.0)
                nc.vector.memset(h2row[:, 66:T2W], 0.0)
                cc_writes.append(nc.sync.dma_start(
                    out=cc_in[r0:r0 + P, :], in_=h2row[:]))

            # ------------- phase 3: share layer-2 node table -------------
            nc.gpsimd.load_library(library_config.standard)
            cc = nc.gpsimd.collective_compute(
                "AllGather", ALU.bypass, replica_groups=groups,
                ins=[cc_in[:]], outs=[t2[:]])
            for w in cc_writes:
                _dep(cc, w, "allgather after cc writes")
            j2tile = cpool.tile([1, 1], F32, tag="j2")
            j2 = nc.gpsimd.memset(j2tile[:], 0.0)
            _dep(j2, cc, "layer2 gathers after allgather")
            nc.gpsimd.load_library(library_config.mlp)

            # ------------- phase 4: layer-2 edge aggregation -------------
            for b in range(BPC):
                r0 = b * P
                tlo = bpool.tile([P, MLO * 4], I32, tag="tlo2")
                nc.sync.dma_start(out=tlo[:], in_=eidx[b, :, cfg.o_lo:cfg.o_hi])
                thi = bpool.tile([P, MHI * 4], I32, tag="thi2")
                nc.sync.dma_start(out=thi[:], in_=eidx[b, :, cfg.o_hi:cfg.o_ad])
                tad = bpool.tile([P, MB * 4], I32, tag="tad2")
                nc.sync.dma_start(out=tad[:], in_=eidx[b, :, cfg.o_ad:cfg.o_dr])
                tdr = bpool.tile([P, MB], I32, tag="tdr2")
                nc.sync.dma_start(out=tdr[:], in_=eidx[b, :, cfg.o_dr:cfg.K])
                ilo = tlo[:].bitcast(I16)
                ihi = thi[:].bitcast(I16)
                iad = tad[:].bitcast(I16)
                drel = tdr[:].bitcast(F32)

                nlo_c = cfg.slot_lo[b]
                nhi_c = cfg.slot_hi[b]
                glist = list(range(nlo_c)) + list(range(MLO, MLO + nhi_c))
                gath2 = bpool.tile([P, MB * T2W], F16, tag="gath2")
                _gpieces(nc, _dep, gath2, 0, T2W, t2[0:LO, :], ilo, nlo_c,
                         j2, "lo gather after table2")
                _gpieces(nc, _dep, gath2, MLO, T2W, t2[LO:NPAD, :], ihi, nhi_c,
                         j2, "hi gather after table2")
                adst2 = bpool.tile([P, MB * T2W], F16, tag="adst2")
                _gpieces(nc, _dep, adst2, 0, T2W, cc_in[:], iad, nlo_c,
                         j2, "adst2 gather after cc writes")
                _gpieces(nc, _dep, adst2, MLO, T2W, cc_in[:],
                         iad[:, MLO * 8:], nhi_c,
                         j2, "adst2 gather after cc writes")

                qv = gath2[:].rearrange("p (m w) -> p m w", m=MB)
                av2 = bpool.tile([P, MB], F16, tag="av2")
                nc.vector.scalar_tensor_tensor(
                    out=av2[:].rearrange("p (m o) -> p m o", m=MB),
                    in0=qv[:, :, 64:65], scalar=shifts_s[:, 1:2],
                    in1=adst2[:].rearrange("p (m w) -> p m w",
                                           m=MB)[:, :, 65:66],
                    op0=ALU.add, op1=ALU.add)
                lk2 = bpool.tile([P, MB], F16, tag="lk2")
                nc.vector.scalar_tensor_tensor(
                    out=lk2[:], in0=av2[:], scalar=NEG_SLOPE, in1=av2[:],
                    op0=ALU.mult, op1=ALU.max)
                e2full = bpool.tile([P, MB * C2], F16, tag="e2full")
                nc.scalar.activation(
                    out=e2full[:],
                    in_=lk2[:].rearrange("p (m o) -> p m o", m=MB)
                           .to_broadcast([P, MB, 1, C2]),
                    func=AF.Exp, bias=shifts_s[:, 0:1])
                wm2 = bpool.tile([P, MB * C2], F16, tag="wm2")
                nc.vector.tensor_tensor(
                    out=wm2[:].rearrange("p (m c) -> p m c", m=MB),
                    in0=qv[:, :, 0:C2],
                    in1=e2full[:].rearrange("p (m c) -> p m c", m=MB),
                    op=ALU.mult)

                pacc2 = ps.tile([P, 272], F32, space="PSUM", tag="acc")
                pmsg2 = pacc2[:, 0:C2]
                pdent2 = ps.tile([P, H1], F32, space="PSUM", tag="den")
                pden2 = pdent2[:, 0:1]
                for gi, g in enumerate(glist):
                    S = spool.tile([P, P], F16, tag="S")
                    nc.vector.tensor_scalar(
                        out=S[:], in0=iota_s[:], scalar1=drel[:, g:g + 1],
                        scalar2=None, op0=ALU.is_equal)
                    nc.tensor.matmul(out=pmsg2, lhsT=S[:],
                                     rhs=wm2[:, g * C2:(g + 1) * C2],
                                     start=(gi == 0),
                                     stop=(gi == len(glist) - 1))
                    nc.tensor.matmul(out=pden2, lhsT=S[:],
                                     rhs=e2full[:, g * C2:g * C2 + 1],
                                     start=(gi == 0),
                                     stop=(gi == len(glist) - 1))

                den2 = opool.tile([P, 1], F32, tag="den2")
                nc.vector.tensor_scalar_add(out=den2[:], in0=pden2,
                                            scalar1=EPS)
                rec2 = opool.tile([P, 1], F32, tag="rec2")
                nc.vector.reciprocal(out=rec2[:], in_=den2[:])
                o2 = opool.tile([P, C2], F32, tag="o2")
                nc.vector.scalar_tensor_tensor(
                    out=o2[:], in0=pmsg2, scalar=rec2[:, 0:1],
                    in1=b2b_s[:], op0=ALU.mult, op1=ALU.add)
                nc.sync.dma_start(out=out[r0:r0 + P, :], in_=o2[:])

    nc.compile()
    return nc


def _wrap16(idx, nid):
    """Pack an int16 index list (len nid) into a [128, nid//16] tile:
    element k at (k%16, k//16), replicated to partitions 16..127."""
    a = np.asarray(idx, np.int16).reshape(nid // 16, 16).T  # [16, nid//16]
    return np.tile(a, (8, 1))


def host_prep(cfg, edge_index):
    n = cfg.n
    src = np.asarray(edge_index[0]).astype(np.int64)
    dst = np.asarray(edge_index[1]).astype(np.int64)
    loop = np.arange(n, dtype=np.int64)
    src = np.concatenate([src, loop])
    dst = np.concatenate([dst, loop])

    order = np.argsort(dst, kind="stable")
    ss = src[order]
    ds = dst[order]
    blk = ds >> 7

    MLO, MHI, MB = cfg.mlo, cfg.mhi, cfg.mb
    NBLK, BPC = cfg.nblk, cfg.bpc
    eidx = np.zeros((NBLK, P, cfg.K), dtype=np.int32)

    starts = np.zeros(NBLK + 1, dtype=np.int64)
    np.cumsum(np.bincount(blk, minlength=NBLK), out=starts[1:])

    for B in range(NBLK):
        s_b = ss[starts[B]:starts[B + 1]]
        d_b = ds[starts[B]:starts[B + 1]]
        lo_m = s_b < LO
        s_lo, d_lo = s_b[lo_m], d_b[lo_m]
        s_hi, d_hi = s_b[~lo_m], d_b[~lo_m]
        nlo, nhi = len(s_lo), len(s_hi)
        assert -(-nlo // P) <= MLO and -(-nhi // P) <= MHI, (B, nlo, nhi)
        cbase = (B // BPC) * BPC * P

        ilo = np.zeros(MLO * P, np.int16)
        ilo[:nlo] = s_lo.astype(np.int16)
        ihi = np.zeros(MHI * P, np.int16)
        ihi[:nhi] = (s_hi - LO).astype(np.int16)
        kidx = np.concatenate([np.arange(nlo), MLO * P + np.arange(nhi)])
        d_all = np.concatenate([d_lo, d_hi])
        iad = np.zeros(MB * P, np.int64)
        iad[kidx] = d_all - cbase
        drel = np.full((P, MB), -1.0, dtype=np.float32)
        drel[kidx % P, kidx // P] = (d_all - (B << 7)).astype(np.float32)

        eidx[B, :, cfg.o_lo:cfg.o_hi] = np.ascontiguousarray(
            _wrap16(ilo, MLO * P)).view(np.int32)
        eidx[B, :, cfg.o_hi:cfg.o_ad] = np.ascontiguousarray(
            _wrap16(ihi, MHI * P)).view(np.int32)
        eidx[B, :, cfg.o_ad:cfg.o_dr] = np.ascontiguousarray(
            _wrap16(iad.astype(np.int16), MB * P)).view(np.int32)
        eidx[B, :, cfg.o_dr:cfg.K] = drel.view(np.int32)

    return [np.ascontiguousarray(eidx[c * BPC:(c + 1) * BPC])
            for c in range(cfg.ncores)]


def compute_m(n, edge_index):
    src = np.asarray(edge_index[0]).astype(np.int64)
    dst = np.asarray(edge_index[1]).astype(np.int64)
    loop = np.arange(n, dtype=np.int64)
    src = np.concatenate([src, loop])
    dst = np.concatenate([dst, loop])
    blk = dst >> 7
    nblk = -(-n // P)
    lo = src < LO
    clo = np.bincount(blk[lo], minlength=nblk)
    chi = np.bincount(blk[~lo], minlength=nblk)
    return int(-(-clo.max() // P)), int(-(-chi.max() // P))


def make_in_maps(cfg, x, W1, att_src1, att_dst1, bias1, W2, att_src2,
                 att_dst2, bias2, edge_index):
    H1, CH1, HC1, C2 = cfg.h1, cfg.ch1, cfg.hc1, cfg.c2
    x = np.asarray(x, dtype=np.float32)
    xpad = np.zeros((cfg.npad, cfg.c_in), dtype=np.float32)
    xpad[: cfg.n] = x
    xt = np.ascontiguousarray(xpad.T).astype(np.float16)

    W1 = np.asarray(W1, np.float32)
    W2 = np.asarray(W2, np.float32)
    as1 = np.asarray(att_src1, np.float32)
    ad1 = np.asarray(att_dst1, np.float32)
    as2 = np.asarray(att_src2, np.float32).reshape(-1)
    ad2 = np.asarray(att_dst2, np.float32).reshape(-1)

    A1s = np.zeros((HC1, H1), dtype=np.float32)
    A1d = np.zeros((HC1, H1), dtype=np.float32)
    hh = np.repeat(np.arange(H1), CH1)
    A1s[np.arange(HC1), hh] = as1.reshape(-1)
    A1d[np.arange(HC1), hh] = ad1.reshape(-1)
    w1aug = np.concatenate([W1, W1 @ A1s, W1 @ A1d], axis=1).astype(np.float16)
    w2aug = np.concatenate([W2, (W2 @ as2)[:, None], (W2 @ ad2)[:, None]],
                           axis=1).astype(np.float16)

    colsum = W2.sum(axis=0)
    c0 = float(colsum @ (as2 + ad2))
    shifts = np.zeros((P, 2), dtype=np.float32)
    shifts[:, 0] = -2.0   # exp bias (cancels in softmax; keeps fp16 safe)
    shifts[:, 1] = -c0    # undo eluplus fold's logit shift (pre-leaky)

    b1b = np.tile(np.asarray(bias1, np.float32).reshape(1, -1),
                  (P, 1)).astype(np.float16)
    b2b = np.tile((np.asarray(bias2, np.float32).reshape(-1) - colsum
                   ).reshape(1, -1), (P, 1)).astype(np.float32)
    iota = np.tile(np.arange(P, dtype=np.float16), (P, 1))
    ident = np.eye(P, dtype=np.float16)

    per_core = host_prep(cfg, edge_index)
    G0 = cfg.nloc // (8 * P)
    in_maps = []
    for c in range(cfg.ncores):
        m = {"xt": xt, "w1aug": w1aug, "w2aug": w2aug, "b1b": b1b,
             "b2b": b2b, "iota": iota, "ident": ident, "shifts": shifts}
        base = c * cfg.bpc * P
        cols = np.minimum(base + np.arange(G0) * 8 * P,
                          cfg.npad - 8 * P).astype(np.int64)
        xoff = (np.arange(P)[:, None] * cfg.npad +
                cols[None, :]).astype(np.int32)
        m["xoff"] = xoff
        m["eidx"] = per_core[c]
        in_maps.append(m)
    return in_maps


_prog_cache = {}
_last_results = None


def kernel(x, edge_index, edge_weight, W1, att_src1, att_dst1, bias1,
           W2, att_src2, att_dst2, bias2):
    global _last_results
    n = x.shape[0]
    # edge_weight is unused (GATConv with edge_dim=None ignores it)
    mlo, mhi = compute_m(n, edge_index)
    mlo, mhi = max(mlo, 13), max(mhi, 8)

    # per-block-slot live chunk counts (max over the 8 cores)
    srcv = np.asarray(edge_index[0]).astype(np.int64)
    dstv = np.asarray(edge_index[1]).astype(np.int64)
    loop = np.arange(n, dtype=np.int64)
    srcv = np.concatenate([srcv, loop])
    dstv = np.concatenate([dstv, loop])
    blkv = dstv >> 7
    nblk = -(-n // P) 
    bpc = -(-n // (P * NCORES))
    clo = np.bincount(blkv[srcv < LO], minlength=NCORES * bpc)
    chi = np.bincount(blkv[srcv >= LO], minlength=NCORES * bpc)
    lo_ch = -(-clo // P)
    hi_ch = -(-chi // P)
    slot_lo = [int(lo_ch[b::bpc].max()) for b in range(bpc)]
    slot_hi = [int(hi_ch[b::bpc].max()) for b in range(bpc)]

    cfg = Cfg(n, mlo, mhi, c_in=x.shape[1], h1=8, ch1=32, c2=64,
              slot_lo=slot_lo, slot_hi=slot_hi)
    key = (cfg.n, cfg.c_in, cfg.mlo, cfg.mhi,
           tuple(slot_lo), tuple(slot_hi))
    if key not in _prog_cache:
        _prog_cache[key] = build_program(cfg)
    nc = _prog_cache[key]

    in_maps = make_in_maps(cfg, x, W1, att_src1, att_dst1, bias1, W2,
                           att_src2, att_dst2, bias2, edge_index)
    res = run_bass_kernel_spmd(nc, in_maps, list(range(cfg.ncores)))
    _last_results = res
    outs = [res.results[c]["out"] for c in range(cfg.ncores)]
    full = np.concatenate(outs, axis=0)[: cfg.n]
    return np.ascontiguousarray(full)
